# revision 4
# baseline (speedup 1.0000x reference)
"""Blockwise (compressed-KV) attention on 8 Trainium2 NeuronCores.

Per-core dataflow (8 heads/core, all HBM I/O bf16):
  q:   DMA-transpose load (xbar) -> qT [d, t] in SBUF, true t order.
  k,v: contiguous loads [p=t//32, a=t%32, d] (8 KiB runs); compression via
       identity-stationary accumulating matmuls: cmp[c,d] = sum_a x[:,a,:].
       v_cmp evacuated with 1/32 scale (bf16); k side transposed once to
       k_cmpT [d,c] (1/32 folded into the exp scale).
  per 512-row sub:
       S[t,c]: 4 matmuls (qT tile stationary, k_cmpT moving)
       exp on ACT -> score slots (bf16, unnormalized)
       row sums: DVE segmented reduce; reciprocal on DVE
       normalize score in place (POOL or DVE)
       score^T: 4 PE transposes of the normalized slots -> PSUM (bf16)
       evac -> w [c,t] bf16;  PV: ONE matmul out^T[d,t] += v_cmp^T w
       (v_cmp stationary per head, w moving N=512); evac out^T -> bf16
  stores (SWDGE/gpsimd ring): score as [p, j, C] blocks, out as out^T
       [D, T]; the host undoes both layouts (pure reshape/transpose).
"""
import math

import numpy as np

import concourse.bass as bass
import concourse.tile as tile
from concourse import mybir
from concourse.bass_utils import run_bass_kernel_spmd
from concourse.vector_clock import ScopedClock

B, H, T, D = 4, 16, 4096, 128
BS_EXPECTED = 32
C = T // BS_EXPECTED  # 128
N_CORES = 8
HEADS_PER_CORE = B * H // N_CORES  # 8
A = 32
N_SUB = 8  # 8 subs x 512 rows
F32 = mybir.dt.float32
BF16 = mybir.dt.bfloat16

# ---------------------------------------------------------------------------
# walrus in this toolchain rejects instructions carrying more than one sync
# wait; hoist extras onto same-engine NOPs.
_MAX_WAITS = 1
_split_counter = [0]


def _split_multi_waits(ordered):
    for insts in ordered.values():
        expanded = []
        for inst in insts:
            si = inst.sync_info
            if si is not None and len(si.on_wait) > _MAX_WAITS:
                waits = list(si.on_wait)
                head, keep = waits[:-_MAX_WAITS], waits[-_MAX_WAITS:]
                for w in head:
                    _split_counter[0] += 1
                    expanded.append(mybir.InstNoOp(
                        name=f"waitsplit_{_split_counter[0]}",
                        ins=[], outs=[],
                        engine=inst.engine,
                        sync_info=mybir.SyncInfo(on_wait=[w], on_update=[]),
                        bass_nofuse=True,
                    ))
                inst.sync_info = mybir.SyncInfo(
                    on_wait=keep, on_update=list(si.on_update)
                )
            expanded.append(inst)
        insts[:] = expanded


_orig_lower_ordered = tile.TileContext._lower_ordered_insts


def _lower_ordered_split(self, ordered):
    _split_multi_waits(ordered)
    return _orig_lower_ordered(self, ordered)


tile.TileContext._lower_ordered_insts = _lower_ordered_split


def _drain_and_barrier_split(self, tick_clock, wait_clock):
    nc = self.nc
    drain_inst = nc.sync.drain()
    wait_clock.add_sem_waits(
        drain_inst.ins, ScopedClock({None: tick_clock.global_clock})
    )
    si = drain_inst.ins.sync_info
    waits = list(si.on_wait) if si is not None else []
    if len(waits) > _MAX_WAITS:
        drain_inst.ins.sync_info = mybir.SyncInfo(
            on_wait=waits[:_MAX_WAITS], on_update=list(si.on_update)
        )
        for i in range(_MAX_WAITS, len(waits), _MAX_WAITS):
            extra = nc.sync.drain()
            extra.ins.sync_info = mybir.SyncInfo(
                on_wait=waits[i : i + _MAX_WAITS], on_update=[]
            )
    nc.all_engine_barrier()
    assert self.sems is not None
    popped = nc._tile_sem_poison_stack.pop()
    assert popped is self._sem_poison
    nc.clear_and_free_semaphores(list(self.sems.allocated().values()))
    nc.all_engine_barrier()


tile.TileContext._drain_and_barrier = _drain_and_barrier_split
# ---------------------------------------------------------------------------


def build_program(reps: int = 1, norm_engine: str = "dve",
                  store_engine: str = "scalar", qt_mode: str = "dma",
                  evac_mode: str = "fixed", norm_bcast: str = "y",
                  sums_mode: str = "head", kv_packed: str = "n",
                  mode: str = "full") -> bass.Bass:
    nc = bass.Bass("TRN2", target_bir_lowering=False, debug=False,
                   num_devices=N_CORES)

    q_d = nc.dram_tensor("q", [HEADS_PER_CORE, T, D], BF16,
                         kind="ExternalInput").ap()
    if kv_packed == "y":
        kv_d = nc.dram_tensor("kv", [HEADS_PER_CORE, 2, T, D], BF16,
                              kind="ExternalInput").ap()
        k_d = v_d = None
    else:
        k_d = nc.dram_tensor("k", [HEADS_PER_CORE, T, D], BF16,
                             kind="ExternalInput").ap()
        v_d = nc.dram_tensor("v", [HEADS_PER_CORE, T, D], BF16,
                             kind="ExternalInput").ap()
    ident_d = nc.dram_tensor("ident", [128, 128], BF16,
                             kind="ExternalInput").ap()
    # scrambled outputs; host fixes layout
    outT_d = nc.dram_tensor("outT", [HEADS_PER_CORE, D, T], BF16,
                            kind="ExternalOutput").ap()
    score_d = nc.dram_tensor("score_scr", [HEADS_PER_CORE, 128, A, C], BF16,
                             kind="ExternalOutput").ap()

    exp_scale = 1.0 / (math.sqrt(D) * BS_EXPECTED)
    inv_bs = 1.0 / BS_EXPECTED

    def store_eng():
        return {"gpsimd": nc.gpsimd, "scalar": nc.scalar,
                "sync": nc.sync}[store_engine]

    with tile.TileContext(nc) as tc:
        with (
            tc.tile_pool(name="singles", bufs=1) as singles,
            tc.tile_pool(name="kv", bufs=4) as kv_pool,
            tc.tile_pool(name="qT", bufs=3) as qT_pool,
            tc.tile_pool(name="heads", bufs=2) as heads,
            tc.tile_pool(name="obuf", bufs=3) as obuf_pool,
            tc.tile_pool(name="w", bufs=3) as w_pool,
            tc.tile_pool(name="small", bufs=4) as small_pool,
            tc.tile_pool(name="psS", bufs=2, space="PSUM") as psS,
            tc.tile_pool(name="psT", bufs=2, space="PSUM") as psT,
            tc.tile_pool(name="psO", bufs=2, space="PSUM") as psO,
            tc.tile_pool(name="psC", bufs=1, space="PSUM") as psC,
        ):
            ident_bf = singles.tile([128, 128], BF16)
            nc.sync.dma_start(out=ident_bf, in_=ident_d)

            for _rep in range(reps):
                for h in range(HEADS_PER_CORE):
                    # ---- loads ------------------------------------------
                    qT_sb = qT_pool.tile([128, T], BF16, tag="qT")
                    if qt_mode == "dma" and mode != "compute":
                        nc.sync.dma_start_transpose(out=qT_sb, in_=q_d[h])
                    if kv_packed == "y":
                        kv_sb = kv_pool.tile([128, 2, A, D], BF16, tag="kp")
                        if mode != "compute":
                            nc.sync.dma_start(
                                out=kv_sb,
                                in_=kv_d[h].rearrange(
                                    "x (p a) d -> p x a d", p=128),
                            )
                        k_sb, v_sb = kv_sb[:, 0], kv_sb[:, 1]
                    else:
                        k_sb = kv_pool.tile([128, A, D], BF16, tag="kv")
                        v_sb = kv_pool.tile([128, A, D], BF16, tag="kv")
                        if mode != "compute":
                            nc.sync.dma_start(
                                out=k_sb,
                                in_=k_d[h].rearrange("(p a) d -> p a d", p=128),
                            )
                            nc.sync.dma_start(
                                out=v_sb,
                                in_=v_d[h].rearrange("(p a) d -> p a d", p=128),
                            )
                    if mode == "dma":
                        score_buf = obuf_pool.tile([128, A, C], BF16, tag="sc")
                        outT_buf = obuf_pool.tile([128, T], BF16, tag="ot")
                        touch = small_pool.tile([128, 4], BF16, tag="tc")
                        nc.vector.tensor_copy(touch[:, 0:1], k_sb[:, 0, 0:1])
                        nc.vector.tensor_copy(touch[:, 1:2], v_sb[:, 0, 0:1])
                        # (packed or split handled identically via slices)
                        nc.vector.tensor_copy(touch[:, 2:3], qT_sb[:, 0:1])
                        nc.vector.memset(score_buf[:, 0:1, 0:1], 0.5)
                        nc.vector.memset(outT_buf[:, 0:1], 0.25)
                        store_eng().dma_start(out=score_d[h], in_=score_buf)
                        store_eng().dma_start(out=outT_d[h], in_=outT_buf)
                        continue

                    # ---- compression: 8 wide accumulating matmuls into
                    # partials [c, aa, d], then a strided DVE reduce -------
                    k_tmp = heads.tile([128, D], BF16, tag="kc")  # [c,d]*32
                    v_cmp = heads.tile([128, D], BF16, tag="vc")  # [c,d]*32
                    for src, dst in ((k_sb, k_tmp), (v_sb, v_cmp)):
                        cmp_ps = psC.tile([128, 4, 128], F32, tag="c")
                        flat = cmp_ps.rearrange("c aa d -> c (aa d)")
                        for g in range(8):
                            nc.tensor.matmul(
                                flat, lhsT=ident_bf,
                                rhs=src[:, 4 * g : 4 * (g + 1), :].rearrange(
                                    "p a d -> p (a d)"),
                                start=(g == 0), stop=(g == 7),
                            )
                        with nc.allow_low_precision(
                                reason="4-way partial sum to bf16; "
                                "tolerance 2e-2"):
                            nc.vector.reduce_sum(
                                dst, cmp_ps.rearrange("c aa d -> c d aa"),
                                axis=mybir.AxisListType.X,
                            )
                    kT_ps = psC.tile([128, 128], BF16, tag="ct")
                    nc.tensor.transpose(kT_ps, k_tmp, ident_bf)
                    k_cmpT = heads.tile([128, C], BF16, tag="kt")  # [d,c]
                    nc.scalar.copy(k_cmpT, kT_ps)

                    score_buf = obuf_pool.tile([128, A, C], BF16, tag="sc")
                    outT_buf = obuf_pool.tile([128, T], BF16, tag="ot")

                    # ---- main loop --------------------------------------
                    def do_qk_exp(s):
                        s_ps = psS.tile([128, 512], F32, tag="s")
                        for j in range(4):
                            nc.tensor.matmul(
                                s_ps[:, 128 * j : 128 * (j + 1)],
                                lhsT=qT_sb[:, 512 * s + 128 * j :
                                           512 * s + 128 * (j + 1)],
                                rhs=k_cmpT,
                                start=True, stop=True,
                            )
                        score_slots = score_buf.rearrange(
                            "p a c -> p (a c)")[:, 512 * s : 512 * (s + 1)]
                        nc.scalar.activation(
                            score_slots, s_ps,
                            mybir.ActivationFunctionType.Exp,
                            scale=exp_scale,
                        )

                    def do_norm(s, recip):
                        neng = (nc.gpsimd if norm_engine == "pool"
                                else nc.vector)
                        if norm_bcast == "y":
                            slots4 = score_buf[:, 4 * s : 4 * (s + 1), :]
                            neng.tensor_tensor(
                                slots4, slots4,
                                recip[:, :, None].broadcast_to((128, 4, C)),
                                op=mybir.AluOpType.mult,
                            )
                        else:
                            for j in range(4):
                                neng.tensor_scalar_mul(
                                    score_buf[:, 4 * s + j, :],
                                    score_buf[:, 4 * s + j, :],
                                    recip[:, j : j + 1],
                                )

                    def do_pv(s):
                        cols = slice(512 * s, 512 * (s + 1))
                        w_ps = psT.tile([128, 512], BF16, tag="w")
                        for j in range(4):
                            nc.tensor.transpose(
                                w_ps[:, 128 * j : 128 * (j + 1)],
                                score_buf[:, 4 * s + j, :], ident_bf,
                            )
                        w_sb = w_pool.tile([128, 512], BF16, tag="w")
                        if evac_mode == "alt" and s % 2 == 0:
                            nc.scalar.copy(w_sb, w_ps)
                        else:
                            nc.vector.tensor_copy(w_sb, w_ps)
                        o_ps = psO.tile([128, 512], F32, tag="o")
                        nc.tensor.matmul(
                            o_ps, lhsT=v_cmp, rhs=w_sb,
                            start=True, stop=True,
                        )
                        if evac_mode == "alt" and s % 2 == 0:
                            nc.vector.tensor_scalar_mul(
                                outT_buf[:, cols], o_ps, inv_bs)
                        else:
                            nc.scalar.activation(
                                outT_buf[:, cols], o_ps,
                                mybir.ActivationFunctionType.Copy,
                                scale=inv_bs,
                            )

                    if sums_mode == "head":
                        for s in range(N_SUB):
                            do_qk_exp(s)
                        sums_h = small_pool.tile([128, A], F32, tag="smh")
                        nc.vector.reduce_sum(
                            sums_h, score_buf, axis=mybir.AxisListType.X)
                        recip_h = small_pool.tile([128, A], F32, tag="rch")
                        nc.vector.reciprocal(recip_h, sums_h)
                        for s in range(N_SUB):
                            do_norm(s, recip_h[:, 4 * s : 4 * (s + 1)])
                            do_pv(s)
                    else:
                        for s in range(N_SUB):
                            do_qk_exp(s)
                            sums = small_pool.tile([128, 4], F32, tag="sm")
                            nc.vector.reduce_sum(
                                sums,
                                score_buf[:, 4 * s : 4 * (s + 1), :],
                                axis=mybir.AxisListType.X,
                            )
                            recip = small_pool.tile([128, 4], F32, tag="rc")
                            nc.vector.reciprocal(recip, sums)
                            do_norm(s, recip)
                            do_pv(s)

                    # ---- stores -----------------------------------------
                    if mode != "compute":
                        store_eng().dma_start(out=score_d[h], in_=score_buf)
                        store_eng().dma_start(out=outT_d[h], in_=outT_buf)
    return nc


def _make_const_inputs():
    import ml_dtypes
    return np.eye(128, dtype=ml_dtypes.bfloat16)


def make_in_maps(q, k, v, kv_packed: str = "n"):
    """q,k,v: [B*H, T, D] float32 -> per-core input dicts (bf16)."""
    import ml_dtypes
    qb = q.astype(ml_dtypes.bfloat16)
    kb = k.astype(ml_dtypes.bfloat16)
    vb = v.astype(ml_dtypes.bfloat16)
    ident = _make_const_inputs()
    kvb = np.stack([kb, vb], axis=1) if kv_packed == "y" else None
    in_maps = []
    for i in range(N_CORES):
        sl = slice(i * HEADS_PER_CORE, (i + 1) * HEADS_PER_CORE)
        m = {"q": qb[sl], "ident": ident}
        if kv_packed == "y":
            m["kv"] = kvb[sl]
        else:
            m["k"] = kb[sl]
            m["v"] = vb[sl]
        in_maps.append(m)
    return in_maps


def unscramble(res_list):
    """res_list: per-core dicts with outT [8,D,T] bf16, score_scr
    [8,128,A,C] bf16 -> (out [B,H,T,D] f32, score [B,H,T,C] f32)."""
    out = np.empty((B * H, T, D), dtype=np.float32)
    score = np.empty((B * H, T, C), dtype=np.float32)
    for i, res in enumerate(res_list):
        sl = slice(i * HEADS_PER_CORE, (i + 1) * HEADS_PER_CORE)
        ot = np.asarray(res["outT"], dtype=np.float32)  # [8, D, T]
        out[sl] = np.swapaxes(ot, 1, 2)
        sc = np.asarray(res["score_scr"], dtype=np.float32)  # [8,128,A,C]
        # score row t = 128*slot + p lives at [p, slot]
        score[sl] = np.swapaxes(sc, 1, 2).reshape(HEADS_PER_CORE, T, C)
    return out.reshape(B, H, T, D), score.reshape(B, H, T, C)


_PROGRAM_CACHE: dict[int, bass.Bass] = {}


def kernel(q: np.ndarray, k: np.ndarray, v: np.ndarray, BS) -> tuple:
    assert int(BS) == BS_EXPECTED, f"kernel hardcodes BS=32, got {BS}"
    q = np.ascontiguousarray(np.asarray(q, dtype=np.float32)).reshape(B * H, T, D)
    k = np.ascontiguousarray(np.asarray(k, dtype=np.float32)).reshape(B * H, T, D)
    v = np.ascontiguousarray(np.asarray(v, dtype=np.float32)).reshape(B * H, T, D)

    if 1 not in _PROGRAM_CACHE:
        _PROGRAM_CACHE[1] = build_program(reps=1)
    nc = _PROGRAM_CACHE[1]
    in_maps = make_in_maps(q, k, v)
    res = run_bass_kernel_spmd(nc, in_maps, core_ids=list(range(N_CORES)))
    return unscramble(res.results)
